# revision 2
# baseline (speedup 1.0000x reference)
"""Trainium2 Bass kernel: Performer (linear) attention + in/out projections.

Problem nn_LinearPerformerAttention_6717328851263:
  x:(4,4096,1024) f32, w_qkv:(1024,3072), proj_matrix:(16,64,256),
  w_out:(1024,1024), b_out:(1024,)

  qkv = x @ w_qkv ; split q,k,v ; per (b,h): q_proj=elu1(q@P_h), k_proj=elu1(k@P_h)
  kv = k_proj^T v ; k_sum = sum_n k_proj ; attn = (q_proj @ kv) / (q_proj@k_sum)
  out = attn @ w_out + b_out

Sharding over 8 cores: core c -> (batch b=c//2, head-group g=c%2: 8 of 16 heads).
Each core computes partial y_c = attn(b, heads_g) @ w_out[512g:512g+512, :].
Host gather: out[b] = y_(b,0) + y_(b,1) + b_out.

v2: all matmul data in fp16 (PE at 2x the fp32r rate; fp16's 10 mantissa
bits keep the chained-matmul error ~0.1%, well inside the 2e-2 gate).
PSUM accumulation stays fp32 everywhere.

Device algorithm per core:
  pass A (per 512-token group): qT,kT = Wq^T xT, Wk^T xT ; v = x Wv
    qT spilled to DRAM scratch (fp16).  k_proj = elu1 of (kT_h^T proj_h)
    computed per (pair, tp) into a 2-bank PSUM tile; kv state accumulated
    with FLIPPED matmuls: lhsT=k_proj-slab [tok, Fchunk], rhs=[v_h|1]
    [tok, 65] -> kv slab [Fchunk, 65] (col 64 = k_sum for free).  This
    yields the [F, hd] layout pass B needs directly -- no PE transposes.
  fixup: kvS[h][s] [128,128] fp16 = kv halves zero-padded (pair-packed
    attn accumulation), ksr[h][s] = k_sum broadcast along free dim via
    one activation(Copy, scale=k_sum_column) per half.
  pass B (per 512-token group): q_projT_h = elu1(proj_h^T qT_h)
    attnT(pair-packed [128,512]) = kvS^T q_projT ; denom = ksr^T q_projT
    z = 1/denom (fast approx) ; attnT *= z ; y = attnT^T @ w_out -> DRAM
elu1(x) = elu(x)+1 = min(exp(x),1) + relu(x):
  E = exp(x) on Scalar (the only exp engine), R = relu(x) on Scalar or
  DVE (split to balance), combine = (E min 1) + R as one fp16 STT on DVE
  (16-bit 2x mode).  All DMA issues ride the GpSimd queue (GpSimd has no
  PSUM port so it can't help with compute; its queue is free).
"""

import numpy as np
from contextlib import ExitStack

import concourse.bass as bass
import concourse.bacc as bacc
import concourse.tile as tile
from concourse import mybir
from concourse.bass_utils import run_bass_kernel_spmd

FP32 = mybir.dt.float32
F16 = mybir.dt.float16
AL = mybir.AluOpType
AF = mybir.ActivationFunctionType

B, SEQ, D = 4, 4096, 1024
H, HD, F = 16, 64, 256
HPC = 8            # heads per core
DH = HPC * HD      # 512 head-space dims per core
P = 128
NCORES = 8


def _emit(tc, n, xT, wq, wk, wv, proj, wout, y, qTd):
    nc = tc.nc
    NG = n // 512       # token groups
    TPG = 4             # 128-token tiles per group

    ctx = ExitStack()
    with ctx:
        const = ctx.enter_context(tc.tile_pool(name="const", bufs=1))

        ones_sb = const.tile([P, P], F16, tag="ones_sb", name="ones_sb")
        nc.vector.memset(ones_sb, 1.0)

        # proj, pair-packed [128, 256]: head 2i at partitions 0:64, head
        # 2i+1 at 64:128 (lhsT/rhs partition bases always match).
        proj_pair = [const.tile([P, F], F16, tag=f"projp{i}", name=f"projp{i}")
                     for i in range(4)]
        for i in range(4):
            nc.gpsimd.dma_start(out=proj_pair[i], in_=proj[i * P:(i + 1) * P, :])

        # attn lhsT, zero-padded to M=128 so a head pair accumulates into one
        # [128,512] PSUM tile (matmul outputs must start at partition 0):
        # kvS[h][s]: [F-chunk 128, 128]; head-half cols (h%2)*64.. hold kv_h,
        # the other 64 cols are zero.  ksr[h][s] same but k_sum replicated,
        # so pair denominators land on matching partitions of one PSUM tile.
        kvS = [[const.tile([P, P], F16, tag=f"kvS{h}_{s}", name=f"kvS{h}_{s}")
                for s in range(2)] for h in range(HPC)]
        ksr = [[const.tile([P, P], F16, tag=f"ksr{h}_{s}", name=f"ksr{h}_{s}")
                for s in range(2)] for h in range(HPC)]
        for h in range(HPC):
            for s in range(2):
                nc.vector.memset(kvS[h][s], 0.0)
                nc.vector.memset(ksr[h][s], 0.0)

        # ---------------- pass A ----------------
        with ExitStack() as actx:
            # kv state accumulators, [128 F(2 chunks), 260] fp32 per pair:
            # col j*130 + s*65 + (0:64) = kv of head 2hp+j, F-chunk s;
            # col j*130 + s*65 + 64 = its k_sum.
            kvaccp = actx.enter_context(tc.tile_pool(name="kvaccp", bufs=1))
            kv_acc = [kvaccp.tile([P, 260], FP32, tag=f"kva{i}", name=f"kva{i}")
                      for i in range(4)]
            wpool = actx.enter_context(tc.tile_pool(name="wpool", bufs=1))
            wq_sb = [wpool.tile([P, DH], F16, tag=f"wq{s}", name=f"wq{s}") for s in range(8)]
            wk_sb = [wpool.tile([P, DH], F16, tag=f"wk{s}", name=f"wk{s}") for s in range(8)]
            wv_sb = [wpool.tile([P, DH], F16, tag=f"wv{s}", name=f"wv{s}") for s in range(8)]
            for s in range(8):
                nc.gpsimd.dma_start(out=wq_sb[s], in_=wq[s * P:(s + 1) * P, :])
                nc.gpsimd.dma_start(out=wk_sb[s], in_=wk[s * P:(s + 1) * P, :])
                nc.gpsimd.dma_start(out=wv_sb[s], in_=wv[s * P:(s + 1) * P, :])

            xtpool = actx.enter_context(tc.tile_pool(name="xtpool", bufs=2))
            ktpool = actx.enter_context(tc.tile_pool(name="ktpool", bufs=2))
            qtpool = actx.enter_context(tc.tile_pool(name="qtpool", bufs=3))
            vpool = actx.enter_context(tc.tile_pool(name="vpool", bufs=2))
            elupool = actx.enter_context(tc.tile_pool(name="elupool", bufs=3))
            mmps = actx.enter_context(tc.tile_pool(name="mmps", bufs=2, space="PSUM"))
            kpps = actx.enter_context(tc.tile_pool(name="kpps", bufs=2, space="PSUM"))
            kvps = actx.enter_context(tc.tile_pool(name="kvps", bufs=2, space="PSUM"))

            xT_v = xT.rearrange("(s p) m -> p s m", p=P)

            for g in range(NG):
                g0 = g * 512
                xt = xtpool.tile([P, 8, 512], F16, tag="xt", name="xt")
                nc.gpsimd.dma_start(out=xt, in_=xT_v[:, :, g0:g0 + 512])

                # qT: spill to DRAM scratch (fp16)
                for fs in range(4):
                    ps = mmps.tile([P, 512], FP32, tag="mm", name="mm")
                    for s in range(8):
                        nc.tensor.matmul(
                            ps, lhsT=(wq_sb[s][:, fs * P:(fs + 1) * P]),
                            rhs=(xt[:, s, :]), start=(s == 0), stop=(s == 7))
                    qt_sb = qtpool.tile([P, 512], F16, tag="qt", name="qt")
                    nc.scalar.copy(qt_sb, ps)
                    nc.gpsimd.dma_start(
                        out=qTd[fs * P:(fs + 1) * P, g0:g0 + 512], in_=qt_sb)

                # kT: kept in SBUF for this group
                kt_sb = [ktpool.tile([P, 512], F16, tag=f"kt{fs}", name=f"kt{fs}")
                         for fs in range(4)]
                for fs in range(4):
                    ps = mmps.tile([P, 512], FP32, tag="mm", name="mm")
                    for s in range(8):
                        nc.tensor.matmul(
                            ps, lhsT=(wk_sb[s][:, fs * P:(fs + 1) * P]),
                            rhs=(xt[:, s, :]), start=(s == 0), stop=(s == 7))
                    nc.scalar.copy(kt_sb[fs], ps)

                # v with ones column: vone[p, t, h, 0:64]=v, [..,64]=1
                vone = vpool.tile([P, TPG, HPC, HD + 1], F16, tag="vone", name="vone")
                nc.scalar.copy(
                    vone[:, :, :, HD],
                    ones_sb[:, 0:TPG * HPC].rearrange(
                        "p (t h) -> p t h", t=TPG))
                for t in range(TPG):
                    ps = mmps.tile([P, 512], FP32, tag="mm", name="mm")
                    for s in range(8):
                        nc.tensor.matmul(
                            ps, lhsT=(xt[:, s, t * P:(t + 1) * P]),
                            rhs=(wv_sb[s]), start=(s == 0), stop=(s == 7))
                    nc.scalar.copy(
                        vone[:, t, :, 0:HD],
                        ps.rearrange("p (h e) -> p h e", h=HPC))

                # k_proj + elu1 + kv accumulation.  kp is a 2-bank PSUM tile
                # holding both heads of the pair x both token tiles of tp:
                # col (h%2)*512 + ti*256 + f.  Even/odd head matmuls are
                # adjacent so the PE runs them concurrently in disjoint row
                # groups (K=64 at partition bases 0/64).
                for hp in range(HPC // 2):
                    kPs = []
                    for tp in range(2):
                        kp = kpps.tile([P, 1024], FP32, tag="kp", name="kp")
                        for ti in range(2):
                            t = tp * 2 + ti
                            for j in range(2):
                                hb = j * HD
                                nc.tensor.matmul(
                                    kp[:, j * 512 + ti * F: j * 512 + (ti + 1) * F],
                                    lhsT=(kt_sb[hp][hb:hb + HD,
                                                    t * P:(t + 1) * P]),
                                    rhs=(proj_pair[hp][hb:hb + HD, :]),
                                    start=True, stop=True)
                        kE = elupool.tile([P, 1024], F16, tag="kE", name="kE")
                        kR = elupool.tile([P, 1024], F16, tag="kR", name="kR")
                        kP = elupool.tile([P, 1024], F16, tag="kP", name="kP")
                        nc.scalar.activation(kE, kp, AF.Exp)
                        nc.vector.tensor_scalar_max(kR, kp, 0.0)
                        nc.vector.scalar_tensor_tensor(
                            kP, in0=kE, scalar=1.0, in1=kR,
                            op0=AL.min, op1=AL.add)
                        kPs.append(kP)
                    # kv state, flipped: out [Fchunk, 65] slabs; col 64 = k_sum
                    kv_ps = kvps.tile([P, 260], FP32, tag="kvg", name="kvg")
                    for j in range(2):
                        for s in range(2):
                            co = j * 130 + s * 65
                            for t in range(TPG):
                                nc.tensor.matmul(
                                    kv_ps[:, co:co + 65],
                                    lhsT=(kPs[t // 2][:, j * 512 + (t % 2) * F
                                                      + s * P: j * 512
                                                      + (t % 2) * F + (s + 1) * P]),
                                    rhs=(vone[:, t, 2 * hp + j, :]),
                                    start=(t == 0), stop=(t == TPG - 1),
                                    skip_group_check=True)
                    if g == 0:
                        nc.vector.tensor_copy(kv_acc[hp], kv_ps)
                    else:
                        nc.vector.tensor_tensor(
                            out=kv_acc[hp], in0=kv_ps, in1=kv_acc[hp],
                            op=AL.add)

            # ------- kv fixup: kv_acc -> kvS (cast) / ksr (replicate) -------
            for hp in range(4):
                for j in range(2):
                    h = 2 * hp + j
                    hb = j * HD
                    for s in range(2):
                        co = j * 130 + s * 65
                        nc.vector.tensor_copy(
                            kvS[h][s][:, hb:hb + HD],
                            kv_acc[hp][:, co:co + HD])
                        # ksr[:, hb:hb+64] = k_sum column, broadcast along free
                        nc.scalar.activation(
                            ksr[h][s][:, hb:hb + HD], ones_sb[:, 0:HD],
                            AF.Copy, scale=kv_acc[hp][:, co + HD:co + HD + 1])

        # ---------------- pass B ----------------
        with ExitStack() as bctx:
            wopool = bctx.enter_context(tc.tile_pool(name="wopool", bufs=1))
            wo_sb = [wopool.tile([P, D], F16, tag=f"wo{s}", name=f"wo{s}") for s in range(4)]
            for s in range(4):
                nc.gpsimd.dma_start(out=wo_sb[s], in_=wout[s * P:(s + 1) * P, :])

            qtbpool = bctx.enter_context(tc.tile_pool(name="qtbpool", bufs=2))
            qppool = bctx.enter_context(tc.tile_pool(name="qppool", bufs=6))
            attpool = bctx.enter_context(tc.tile_pool(name="attpool", bufs=2))
            zpool = bctx.enter_context(tc.tile_pool(name="zpool", bufs=2))
            ypool = bctx.enter_context(tc.tile_pool(name="ypool", bufs=3))
            qpps = bctx.enter_context(tc.tile_pool(name="qpps", bufs=3, space="PSUM"))
            atps = bctx.enter_context(tc.tile_pool(name="atps", bufs=2, space="PSUM"))
            dnps = bctx.enter_context(tc.tile_pool(name="dnps", bufs=1, space="PSUM"))
            yps = bctx.enter_context(tc.tile_pool(name="yps", bufs=1, space="PSUM"))

            # pair-packed qT view: pair hp -> partitions 0:64 = head 2hp,
            # 64:128 = head 2hp+1
            qTd_v = qTd.rearrange("(hh p) m -> p hh m", p=P)

            for g in range(NG):
                g0 = g * 512
                qt = qtbpool.tile([P, HPC // 2, 512], F16, tag="qt", name="qt")
                nc.gpsimd.dma_start(out=qt, in_=qTd_v[:, :, g0:g0 + 512])

                att_sb = [attpool.tile([P, 512], F16, tag=f"att{i}", name=f"att{i}")
                          for i in range(4)]
                for hp in range(HPC // 2):
                    aps = atps.tile([P, 512], FP32, tag="at", name="aps")
                    dps = dnps.tile([P, 512], FP32, tag="dn", name="dn")
                    # q_projT + elu1, one [128,512] tile per (head, F-chunk)
                    qP = [[None, None], [None, None]]
                    for s in range(2):
                        pss = []
                        for j in range(2):
                            hb = j * HD
                            ps = qpps.tile([P, 512], FP32, tag="qp", name="qp")
                            nc.tensor.matmul(
                                ps, lhsT=(proj_pair[hp][hb:hb + HD,
                                                        s * P:(s + 1) * P]),
                                rhs=(qt[hb:hb + HD, hp, :]),
                                start=True, stop=True)
                            pss.append(ps)
                        for j in range(2):
                            ps = pss[j]
                            qE = qppool.tile([P, 512], F16, tag="qE", name="qE")
                            qR = qppool.tile([P, 512], F16, tag="qR", name="qR")
                            qPs = qppool.tile([P, 512], F16, tag="qP", name="qP")
                            nc.scalar.activation(qE, ps, AF.Exp)
                            # R: split between Scalar and DVE to balance
                            if (hp + s) % 2 == 0:
                                nc.scalar.activation(qR, ps, AF.Relu)
                            else:
                                nc.vector.tensor_scalar_max(qR, ps, 0.0)
                            nc.vector.scalar_tensor_tensor(
                                qPs, in0=qE, scalar=1.0, in1=qR,
                                op0=AL.min, op1=AL.add)
                            qP[j][s] = qPs

                    for j in range(2):
                        h = 2 * hp + j
                        for s in range(2):
                            first = j == 0 and s == 0
                            last = j == 1 and s == 1
                            nc.tensor.matmul(
                                aps, lhsT=(kvS[h][s]), rhs=(qP[j][s]),
                                start=first, stop=last,
                                skip_group_check=True)
                            nc.tensor.matmul(
                                dps, lhsT=(ksr[h][s]), rhs=(qP[j][s]),
                                start=first, stop=last,
                                skip_group_check=True)
                    # z for both heads at once; evict attnT with z fused.
                    # approx reciprocal (~18 bits) is far below fp16 noise.
                    zb = zpool.tile([P, 512], FP32, tag="zb", name="zb")
                    nc.vector.reciprocal_approx_fast(zb, dps)
                    nc.vector.tensor_tensor(
                        out=att_sb[hp], in0=aps, in1=zb, op=AL.mult)

                # y = attnT^T @ w_out; consecutive o-halves share lhsT
                for t in range(TPG):
                    pso = [yps.tile([P, 512], FP32, tag=f"y{o}", name=f"y{o}")
                           for o in range(2)]
                    for s in range(4):
                        for o in range(2):
                            nc.tensor.matmul(
                                pso[o], lhsT=(att_sb[s][:, t * P:(t + 1) * P]),
                                rhs=(wo_sb[s][:, o * 512:(o + 1) * 512]),
                                start=(s == 0), stop=(s == 3))
                    for o in range(2):
                        y_sb = ypool.tile([P, 512], F16, tag="ysb", name="ysb")
                        if (t + o) % 2 == 0:
                            nc.scalar.copy(y_sb, pso[o])
                        else:
                            nc.vector.tensor_copy(y_sb, pso[o])
                        nc.gpsimd.dma_start(
                            out=y[g0 + t * P: g0 + (t + 1) * P,
                                  o * 512:(o + 1) * 512],
                            in_=y_sb)


def build(n=SEQ):
    # Bacc (not raw Bass): its compile pipeline splits multi-waits into
    # event semaphores (TRN2 allows at most 1 sync wait per instruction).
    nc = bacc.Bacc("TRN2", target_bir_lowering=False, debug=False,
                   enable_asserts=False)
    xT = nc.declare_dram_parameter("xT", [D, n], F16, isOutput=False)
    wq = nc.declare_dram_parameter("wq", [D, DH], F16, isOutput=False)
    wk = nc.declare_dram_parameter("wk", [D, DH], F16, isOutput=False)
    wv = nc.declare_dram_parameter("wv", [D, DH], F16, isOutput=False)
    proj = nc.declare_dram_parameter("proj", [DH, F], F16, isOutput=False)
    wout = nc.declare_dram_parameter("wout", [DH, D], F16, isOutput=False)
    y = nc.declare_dram_parameter("y", [n, D], F16, isOutput=True)
    qTd = nc.dram_tensor("qT_scratch", [DH, n], F16)
    with tile.TileContext(nc) as tc:
        _emit(tc, n, xT, wq, wk, wv, proj, wout, y, qTd)
    nc.finalize()
    return nc


def make_in_maps(x, w_qkv, proj_matrix, w_out):
    x = np.asarray(x, np.float32)
    w_qkv = np.asarray(w_qkv, np.float32)
    proj_matrix = np.asarray(proj_matrix, np.float32)
    w_out = np.asarray(w_out, np.float32)
    in_maps = []
    for c in range(NCORES):
        b, g = c // 2, c % 2
        in_maps.append({
            "xT": x[b].T.astype(np.float16),
            "wq": w_qkv[:, DH * g:DH * (g + 1)].astype(np.float16),
            "wk": w_qkv[:, D + DH * g:D + DH * (g + 1)].astype(np.float16),
            "wv": w_qkv[:, 2 * D + DH * g:2 * D + DH * (g + 1)].astype(np.float16),
            "proj": proj_matrix[HPC * g:HPC * (g + 1)].reshape(DH, F)
                    .astype(np.float16),
            "wout": w_out[DH * g:DH * (g + 1), :].astype(np.float16),
        })
    return in_maps


_NC_CACHE = {}


def get_nc(n=SEQ):
    if n not in _NC_CACHE:
        _NC_CACHE[n] = build(n)
    return _NC_CACHE[n]


def _install_ntff_hook_shim():
    """The agent image's antenv lacks axon_hooks; recreate it so
    run_bass_kernel_spmd(trace=True) can capture NTFF profiles."""
    import sys
    import types
    try:
        from antenv.axon_hooks import get_axon_ntff_profile_hook  # noqa: F401
        return True
    except ImportError:
        pass
    try:
        from trn_agent_boot.trn_boot import _ntff_profile_via_ctypes
        import antenv
        mod = types.ModuleType("antenv.axon_hooks")
        mod._hook = _ntff_profile_via_ctypes("/opt/axon/libaxon_pjrt.so")
        mod.set_axon_ntff_profile_hook = lambda h: setattr(mod, "_hook", h)
        mod.get_axon_ntff_profile_hook = lambda: mod._hook
        sys.modules["antenv.axon_hooks"] = mod
        antenv.axon_hooks = mod
        return True
    except Exception as e:  # profiling is best-effort
        print(f"ntff hook shim failed: {e}")
        return False


def run(x, w_qkv, proj_matrix, w_out, b_out, trace=False, **kw):
    if trace:
        _install_ntff_hook_shim()
    nc = get_nc(SEQ)
    in_maps = make_in_maps(x, w_qkv, proj_matrix, w_out)
    res = run_bass_kernel_spmd(nc, in_maps, list(range(NCORES)),
                               trace=trace, **kw)
    b_out = np.asarray(b_out, np.float32)
    out = np.empty((B, SEQ, D), np.float32)
    for b in range(B):
        out[b] = res.results[2 * b]["y"].astype(np.float32) \
            + res.results[2 * b + 1]["y"].astype(np.float32) \
            + b_out[None, :]
    return out, res


def kernel(x, w_qkv, proj_matrix, w_out, b_out):
    out, _ = run(x, w_qkv, proj_matrix, w_out, b_out)
    return out
